# revision 1
# baseline (speedup 1.0000x reference)
"""GCNNet2 on 8 Trainium2 NeuronCores (Bass/Tile).

Strategy: shard nodes (contiguous 6250-node ranges) across 8 cores; each core
owns the aggregation for its dst range. Per GCN layer: local m = h @ W,
AllGather m into a replicated [50000, 128] DRAM table, dma_gather m[src] for
the core's (dst-sorted) edges, segmented-sum via norm-scaled one-hot
indicator matmuls accumulating in PSUM (self-loop terms seeded with scaled
transpose-matmuls), BatchNorm via a tiny AllReduce of per-core sums, fused
scale/bias/relu + residual. Global mean pool via indicator matmul +
AllReduce, then the MLP readout (replicated).
"""
import numpy as np

# Problem constants (hardcoded per contract; kernel.py must be self-contained)
N = 50000
E = 800000
DIN = 146
D = 128
G = 64
L = 4
NC = 10
EPS = 1e-5

C = 8          # cores
NL = N // C    # 6250 nodes per core
WIN = 128      # aggregation window (PSUM free width)
NWIN = (NL + WIN - 1) // WIN          # 13 windows (12x512 + 106)
WIN_W = [min(WIN, NL - w * WIN) for w in range(NWIN)]
HALF = 25000   # int16 gather-index range per table half
NT = (NL + 127) // 128                # 49 node-tiles per core (48x128 + 106)
NT_W = [min(128, NL - t * 128) for t in range(NT)]
MAX_TILES_PER_CALL = 8
AGG_BF16 = False


def _preprocess(x, edge_index, batch):
    src = np.asarray(edge_index[0], dtype=np.int64)
    dst = np.asarray(edge_index[1], dtype=np.int64)
    batch = np.asarray(batch, dtype=np.int64)

    deg = (np.bincount(dst, minlength=N) + 1).astype(np.float32)  # + self-loop
    dinv = (1.0 / np.sqrt(deg)).astype(np.float32)
    norm = (dinv[src] * dinv[dst]).astype(np.float32)
    norm_self = (dinv * dinv).astype(np.float32)

    core = dst // NL
    wloc = (dst % NL) // WIN
    dstl = ((dst % NL) % WIN).astype(np.float32)
    half = src // HALF
    idx16 = (src % HALF).astype(np.int16)

    # bucket[(c, w, h)] -> edge index list
    key = (core * NWIN + wloc) * 2 + half
    order = np.argsort(key, kind="stable")
    key_s = key[order]
    bounds = np.searchsorted(key_s, np.arange(C * NWIN * 2 + 1))

    # core-invariant tile counts per (w, h)
    T = np.zeros((NWIN, 2), dtype=np.int64)
    for w in range(NWIN):
        for h in range(2):
            mx = 0
            for c in range(C):
                k = (c * NWIN + w) * 2 + h
                mx = max(mx, bounds[k + 1] - bounds[k])
            T[w, h] = -(-mx // 128)  # ceil

    # static tile stream + call list (identical across cores)
    tile_meta = []      # (w, h) per tile
    calls = []          # (h, tile_start, n_tiles, idx_col_off)
    col_off = 0
    for w in range(NWIN):
        for h in range(2):
            t0 = len(tile_meta)
            nt = int(T[w, h])
            tile_meta.extend([(w, h)] * nt)
            r = 0
            while r < nt:
                j = min(MAX_TILES_PER_CALL, nt - r)
                calls.append((h, t0 + r, j, col_off))
                col_off += 8 * j
                r += j
    TILES = len(tile_meta)
    IDXCOLS = col_off

    per_core = []
    for c in range(C):
        idx_arr = np.zeros((16, IDXCOLS), dtype=np.int16)
        dstl_arr = np.zeros((128, TILES), dtype=np.float32)
        norm_arr = np.zeros((128, TILES), dtype=np.float32)
        # fill per (w, h) buckets into their tiles
        t_base = 0
        for w in range(NWIN):
            for h in range(2):
                k = (c * NWIN + w) * 2 + h
                el = order[bounds[k]:bounds[k + 1]]
                nt = int(T[w, h])
                nslots = nt * 128
                tile_idx16 = np.zeros(nslots, dtype=np.int16)
                tile_dstl = np.zeros(nslots, dtype=np.float32)
                tile_norm = np.zeros(nslots, dtype=np.float32)
                tile_idx16[:len(el)] = idx16[el]
                tile_dstl[:len(el)] = dstl[el]
                tile_norm[:len(el)] = norm[el]
                for j in range(nt):
                    dstl_arr[:, t_base + j] = tile_dstl[j * 128:(j + 1) * 128]
                    norm_arr[:, t_base + j] = tile_norm[j * 128:(j + 1) * 128]
                # leave idx packing to the call loop below (needs call offsets)
                if not hasattr(idx_arr, "_tiles"):
                    pass
                t_base += nt
        # pack idx per call (tile-major within call)
        # rebuild flat idx per tile first
        flat_idx = np.zeros((TILES, 128), dtype=np.int16)
        t_base = 0
        for w in range(NWIN):
            for h in range(2):
                k = (c * NWIN + w) * 2 + h
                el = order[bounds[k]:bounds[k + 1]]
                nt = int(T[w, h])
                nslots = nt * 128
                tile_idx16 = np.zeros(nslots, dtype=np.int16)
                tile_idx16[:len(el)] = idx16[el]
                flat_idx[t_base:t_base + nt] = tile_idx16.reshape(nt, 128)
                t_base += nt
        for (h, t0, j, off) in calls:
            seq = flat_idx[t0:t0 + j].reshape(-1)  # i = jj*128 + p
            cols = 8 * j
            idx_arr[:, off:off + cols] = seq.reshape(cols, 16).T
        idx_rep = np.tile(idx_arr, (8, 1))

        ns_p = np.zeros((128, NT), dtype=np.float32)
        bl_p = np.full((128, NT), -1.0, dtype=np.float32)
        for t in range(NT):
            cw = NT_W[t]
            g0 = c * NL + t * 128
            ns_p[:cw, t] = norm_self[g0:g0 + cw]
            bl_p[:cw, t] = batch[g0:g0 + cw].astype(np.float32)

        x_t = np.ascontiguousarray(x[c * NL:(c + 1) * NL].T.astype(np.float32))
        per_core.append(dict(idx=idx_rep, dstl=dstl_arr, norm=norm_arr,
                             x_t=x_t, ns=ns_p, bl=bl_p))

    cnt = np.bincount(batch, minlength=G).astype(np.float32)
    inv_cnt = (1.0 / np.maximum(cnt, 1.0)).astype(np.float32).reshape(G, 1)

    meta = dict(T=T, tile_meta=tile_meta, calls=calls, TILES=TILES,
                IDXCOLS=IDXCOLS)
    return meta, per_core, inv_cnt


def _build(meta):
    import concourse.bacc as bacc
    import concourse.bass as bass
    import concourse.mybir as mybir
    import concourse.tile as tile

    f32 = mybir.dt.float32
    bf16 = mybir.dt.bfloat16
    agdt = bf16 if AGG_BF16 else f32
    i16 = mybir.dt.int16
    Alu = mybir.AluOpType
    Act = mybir.ActivationFunctionType
    Axis = mybir.AxisListType

    TILES = meta["TILES"]
    IDXCOLS = meta["IDXCOLS"]
    tile_meta = meta["tile_meta"]
    calls = meta["calls"]

    import os
    debug = bool(os.environ.get("BASS_KERNEL_DEBUG"))

    nc = bacc.Bacc(None, target_bir_lowering=False, num_swdge_queues=4)

    # ---- parameters ----
    P = {}
    P["x_t"] = nc.declare_dram_parameter("x_t", [DIN, NL], f32, isOutput=False)
    P["idx"] = nc.declare_dram_parameter("idx", [128, IDXCOLS], i16, isOutput=False)
    P["dstl"] = nc.declare_dram_parameter("dstl", [128, TILES], f32, isOutput=False)
    P["norm"] = nc.declare_dram_parameter("norm", [128, TILES], f32, isOutput=False)
    P["ns"] = nc.declare_dram_parameter("ns", [128, NT], f32, isOutput=False)
    P["bl"] = nc.declare_dram_parameter("bl", [128, NT], f32, isOutput=False)
    P["W_emb"] = nc.declare_dram_parameter("W_emb", [DIN, D], f32, isOutput=False)
    P["b_emb"] = nc.declare_dram_parameter("b_emb", [D, 1], f32, isOutput=False)
    P["W_gcn"] = nc.declare_dram_parameter("W_gcn", [L, D, D], f32, isOutput=False)
    P["bgcn_t"] = nc.declare_dram_parameter("bgcn_t", [D, L], f32, isOutput=False)
    P["gamma_t"] = nc.declare_dram_parameter("gamma_t", [D, L], f32, isOutput=False)
    P["beta_t"] = nc.declare_dram_parameter("beta_t", [D, L], f32, isOutput=False)
    P["W_r1"] = nc.declare_dram_parameter("W_r1", [D, D // 2], f32, isOutput=False)
    P["b_r1"] = nc.declare_dram_parameter("b_r1", [D // 2, 1], f32, isOutput=False)
    P["W_r2"] = nc.declare_dram_parameter("W_r2", [D // 2, D // 4], f32, isOutput=False)
    P["b_r2"] = nc.declare_dram_parameter("b_r2", [D // 4, 1], f32, isOutput=False)
    P["W_r3"] = nc.declare_dram_parameter("W_r3", [D // 4, NC], f32, isOutput=False)
    P["b_r3"] = nc.declare_dram_parameter("b_r3", [NC, 1], f32, isOutput=False)
    P["iota"] = nc.declare_dram_parameter("iota", [128, WIN], f32, isOutput=False)
    P["ident"] = nc.declare_dram_parameter("ident", [128, 128], f32, isOutput=False)
    P["inv_cnt"] = nc.declare_dram_parameter("inv_cnt", [G, 1], f32, isOutput=False)
    out_p = nc.declare_dram_parameter("out", [NC, G], f32, isOutput=True)
    if debug:
        dbg_hemb = nc.declare_dram_parameter("dbg_hemb", [128, NL], f32, isOutput=True)
        dbg_hagg0 = nc.declare_dram_parameter("dbg_hagg0", [128, NL], f32, isOutput=True)
        dbg_stat0 = nc.declare_dram_parameter("dbg_stat0", [128, 2], f32, isOutput=True)
        dbg_h0 = nc.declare_dram_parameter("dbg_h0", [128, NL], f32, isOutput=True)

    rg = [list(range(C))]

    with tile.TileContext(nc) as tc:
        with (
            tc.tile_pool(name="const", bufs=1) as cst,
            tc.tile_pool(name="hbuf", bufs=1) as hbuf,
            tc.tile_pool(name="gd", bufs=6) as gd,
            tc.tile_pool(name="st", bufs=4) as stp,
            tc.tile_pool(name="work", bufs=3) as wk,
            tc.tile_pool(name="mstage", bufs=3) as msp,
            tc.tile_pool(name="pag", bufs=3, space="PSUM") as pag,
            tc.tile_pool(name="pmm", bufs=2, space="PSUM") as pmm,
            tc.tile_pool(name="dram", bufs=1, space="DRAM") as drp,
        ):
            # ---- resident SBUF constants/metadata ----
            def load_const(name, shape, dt=f32):
                t = cst.tile(shape, dt, tag=f"c_{name}")
                nc.sync.dma_start(out=t[:], in_=P[name][:])
                return t

            idx_sb = load_const("idx", [128, IDXCOLS], i16)
            dstl_sb = load_const("dstl", [128, TILES])
            norm_sb = load_const("norm", [128, TILES])
            ns_sb = load_const("ns", [128, NT])
            bl_sb = load_const("bl", [128, NT])
            iota_sb = load_const("iota", [128, WIN])
            ident_sb = load_const("ident", [128, 128])
            bgcn_sb = load_const("bgcn_t", [D, L])
            gamma_sb = load_const("gamma_t", [D, L])
            beta_sb = load_const("beta_t", [D, L])
            bemb_sb = load_const("b_emb", [D, 1])
            wr1_sb = load_const("W_r1", [D, D // 2])
            br1_sb = load_const("b_r1", [D // 2, 1])
            wr2_sb = load_const("W_r2", [D // 2, D // 4])
            br2_sb = load_const("b_r2", [D // 4, 1])
            wr3_sb = load_const("W_r3", [D // 4, NC])
            br3_sb = load_const("b_r3", [NC, 1])
            invc_sb = load_const("inv_cnt", [G, 1])

            wemb1 = cst.tile([128, D], f32)
            nc.sync.dma_start(out=wemb1[:], in_=P["W_emb"][0:128, :])
            wemb2 = cst.tile([DIN - 128, D], f32)
            nc.sync.dma_start(out=wemb2[:], in_=P["W_emb"][128:DIN, :])
            wgcn_sb = cst.tile([128, L * D], f32)
            for l in range(L):
                nc.sync.dma_start(out=wgcn_sb[:, l * D:(l + 1) * D],
                                  in_=P["W_gcn"][l])

            hA = hbuf.tile([128, NL], f32)
            hB = hbuf.tile([128, NL], f32)
            hagg = hbuf.tile([128, NL], f32)
            sums = hbuf.tile([128, NWIN], f32)
            sumsq = hbuf.tile([128, NWIN], f32)

            # DRAM internals
            m_bounce = drp.tile([NL, D], agdt)
            m_full = drp.tile([N, D], agdt)
            stat_in = drp.tile([128, 2], f32)
            stat_out = drp.tile([128, 2], f32)
            pool_in = drp.tile([G, D], f32)
            pool_out = drp.tile([G, D], f32)

            # ---- embedding: h0 = x @ W_emb + b_emb (feat-major) ----
            # h0_T = W_emb.T @ x_T
            nchunks = (NL + WIN - 1) // WIN
            for ch in range(nchunks):
                c0 = ch * WIN
                cw = min(WIN, NL - c0)
                x1 = wk.tile([128, WIN], f32, tag="x1")
                nc.sync.dma_start(out=x1[:, :cw], in_=P["x_t"][0:128, c0:c0 + cw])
                x2 = wk.tile([DIN - 128, WIN], f32, tag="x2")
                nc.sync.dma_start(out=x2[:, :cw], in_=P["x_t"][128:DIN, c0:c0 + cw])
                pe = pag.tile([128, WIN], f32, tag="pagg")
                nc.tensor.matmul(out=pe[:, :cw], lhsT=wemb1[:], rhs=x1[:, :cw],
                                 start=True, stop=False)
                nc.tensor.matmul(out=pe[:, :cw], lhsT=wemb2[:], rhs=x2[:, :cw],
                                 start=False, stop=True)
                nc.vector.tensor_scalar(out=hA[:, c0:c0 + cw], in0=pe[:, :cw],
                                        scalar1=bemb_sb[:, 0:1], scalar2=None,
                                        op0=Alu.add)

            hbufs = [hA, hB]
            call_counter = [0]
            if debug:
                nc.sync.dma_start(out=dbg_hemb[:], in_=hA[:])

            # ---- GCN layers ----
            for l in range(L):
                h_in = hbufs[l % 2]
                h_out = hbufs[(l + 1) % 2]
                W_l = wgcn_sb[:, l * D:(l + 1) * D]

                # m rows (node-major) for AG: m = h.T_tile.T @ W
                for t in range(NT):
                    cw = NT_W[t]
                    pm = pmm.tile([128, D], f32, tag="pm")
                    nc.tensor.matmul(out=pm[:cw, :], lhsT=h_in[:, t * 128:t * 128 + cw],
                                     rhs=W_l, start=True, stop=True)
                    ms = msp.tile([128, D], agdt, tag="ms")
                    nc.vector.tensor_copy(out=ms[:cw, :], in_=pm[:cw, :])
                    nc.sync.dma_start(out=m_bounce[t * 128:t * 128 + cw, :],
                                      in_=ms[:cw, :])

                nc.gpsimd.collective_compute(
                    "AllGather", Alu.bypass, replica_groups=rg,
                    ins=[m_bounce.opt()], outs=[m_full.opt()],
                )

                # aggregation per window
                tiles_by_w = {}
                for ti, (w, h) in enumerate(tile_meta):
                    tiles_by_w.setdefault(w, []).append(ti)
                calls_by_w = {}
                for (h, t0, j, off) in calls:
                    w = tile_meta[t0][0]
                    calls_by_w.setdefault(w, []).append((h, t0, j, off))

                for w in range(NWIN):
                    wd = WIN_W[w]
                    pw = pag.tile([128, WIN], f32, tag="pagg")
                    # gathers for this window's edge tiles
                    w_tiles = tiles_by_w.get(w, [])
                    w_calls = calls_by_w.get(w, [])
                    gts = {}
                    for (h, t0, j, off) in w_calls:
                        gt = gd.tile([128, MAX_TILES_PER_CALL, D], agdt, tag="gt")
                        tab = m_full[h * HALF:(h + 1) * HALF, :]
                        nc.gpsimd.dma_gather(
                            gt[:, :j, :], tab,
                            idx_sb[:, off:off + 8 * j],
                            128 * j, 128 * j, D,
                            queue_num=call_counter[0] % 4,
                        )
                        call_counter[0] += 1
                        for jj in range(j):
                            gts[t0 + jj] = (gt, jj)

                    def edge_matmul(ti, start, stop):
                        gt, jj = gts[ti]
                        S = stp.tile([128, WIN], agdt, tag="S")
                        nc.vector.tensor_scalar(
                            out=S[:, :wd], in0=iota_sb[:, :wd],
                            scalar1=dstl_sb[:, ti:ti + 1],
                            scalar2=norm_sb[:, ti:ti + 1],
                            op0=Alu.is_equal, op1=Alu.mult)
                        nc.tensor.matmul(out=pw[:, :wd], lhsT=gt[:, jj, :],
                                         rhs=S[:, :wd], start=start, stop=stop)

                    def seed_matmul(t, start, stop):
                        cw = NT_W[t]
                        mt = msp.tile([128, D], agdt, tag="mseed")
                        nc.sync.dma_start(
                            out=mt[:cw, :],
                            in_=m_bounce[t * 128:t * 128 + cw, :])
                        dg = wk.tile([128, 128], agdt, tag="diag")
                        nc.vector.tensor_scalar(
                            out=dg[:cw, :cw], in0=ident_sb[:cw, :cw],
                            scalar1=ns_sb[:cw, t:t + 1], scalar2=None,
                            op0=Alu.mult)
                        off_c = t * 128 - w * WIN
                        nc.tensor.matmul(out=pw[:, off_c:off_c + cw],
                                         lhsT=mt[:cw, :], rhs=dg[:cw, :cw],
                                         start=start, stop=stop)

                    # Issue order: first edge tile (full window width) opens the
                    # accumulation group with start=True, then the self-loop
                    # seeds, then the remaining edge tiles. start=True resets
                    # the whole PSUM tile's accumulation state, so it must be
                    # the one full-width matmul, issued first.
                    assert w_tiles, f"window {w} has no edge tiles"
                    nt0 = (w * WIN) // 128
                    nt1 = min((w * WIN + wd + 127) // 128, NT)
                    issue = ([("e", w_tiles[0])]
                             + [("s", t) for t in range(nt0, nt1)]
                             + [("e", ti) for ti in w_tiles[1:]])
                    for i, (kind, v) in enumerate(issue):
                        start = i == 0
                        stop = i == len(issue) - 1
                        if kind == "e":
                            edge_matmul(v, start, stop)
                        else:
                            seed_matmul(v, start, stop)
                    # evict + bias + stats
                    w0 = w * WIN
                    nc.vector.tensor_scalar(out=hagg[:, w0:w0 + wd],
                                            in0=pw[:, :wd],
                                            scalar1=bgcn_sb[:, l:l + 1],
                                            scalar2=None, op0=Alu.add)
                    nc.vector.reduce_sum(out=sums[:, w:w + 1],
                                         in_=hagg[:, w0:w0 + wd], axis=Axis.X)
                    sq = wk.tile([128, WIN], f32, tag="sq")
                    nc.scalar.square(out=sq[:, :wd], in_=hagg[:, w0:w0 + wd])
                    nc.vector.reduce_sum(out=sumsq[:, w:w + 1], in_=sq[:, :wd],
                                         axis=Axis.X)

                # global BN stats
                stat_sb = wk.tile([128, 2], f32, tag="stat")
                nc.vector.reduce_sum(out=stat_sb[:, 0:1], in_=sums[:], axis=Axis.X)
                nc.vector.reduce_sum(out=stat_sb[:, 1:2], in_=sumsq[:], axis=Axis.X)
                nc.sync.dma_start(out=stat_in[:], in_=stat_sb[:])
                nc.gpsimd.collective_compute(
                    "AllReduce", Alu.add, replica_groups=rg,
                    ins=[stat_in.opt()], outs=[stat_out.opt()],
                )
                stat_g = wk.tile([128, 2], f32, tag="statg")
                nc.sync.dma_start(out=stat_g[:], in_=stat_out[:])

                mu = wk.tile([128, 1], f32, tag="mu")
                nc.vector.tensor_scalar(out=mu[:], in0=stat_g[:, 0:1],
                                        scalar1=1.0 / N, scalar2=None, op0=Alu.mult)
                ex2 = wk.tile([128, 1], f32, tag="ex2")
                nc.vector.tensor_scalar(out=ex2[:], in0=stat_g[:, 1:2],
                                        scalar1=1.0 / N, scalar2=None, op0=Alu.mult)
                musq = wk.tile([128, 1], f32, tag="musq")
                nc.vector.tensor_tensor(out=musq[:], in0=mu[:], in1=mu[:], op=Alu.mult)
                var = wk.tile([128, 1], f32, tag="var")
                nc.vector.tensor_tensor(out=var[:], in0=ex2[:], in1=musq[:],
                                        op=Alu.subtract)
                var2 = wk.tile([128, 1], f32, tag="var2")
                nc.vector.tensor_scalar(out=var2[:], in0=var[:], scalar1=float(EPS),
                                        scalar2=None, op0=Alu.add)
                stdv = wk.tile([128, 1], f32, tag="stdv")
                nc.scalar.activation(out=stdv[:], in_=var2[:], func=Act.Sqrt)
                rinv = wk.tile([128, 1], f32, tag="rinv")
                nc.vector.reciprocal(out=rinv[:], in_=stdv[:])
                a_t = wk.tile([128, 1], f32, tag="a_t")
                nc.vector.tensor_tensor(out=a_t[:], in0=gamma_sb[:, l:l + 1],
                                        in1=rinv[:], op=Alu.mult)
                t1 = wk.tile([128, 1], f32, tag="t1")
                nc.vector.tensor_tensor(out=t1[:], in0=mu[:], in1=a_t[:], op=Alu.mult)
                b2 = wk.tile([128, 1], f32, tag="b2")
                nc.vector.tensor_tensor(out=b2[:], in0=beta_sb[:, l:l + 1],
                                        in1=t1[:], op=Alu.subtract)

                for w in range(NWIN):
                    wd = WIN_W[w]
                    w0 = w * WIN
                    rl = wk.tile([128, WIN], f32, tag="rl")
                    nc.scalar.activation(out=rl[:, :wd], in_=hagg[:, w0:w0 + wd],
                                         func=Act.Relu, bias=b2[:, 0:1],
                                         scale=a_t[:, 0:1])
                    nc.vector.tensor_add(out=h_out[:, w0:w0 + wd],
                                         in0=rl[:, :wd], in1=h_in[:, w0:w0 + wd])
                if debug and l == 0:
                    nc.sync.dma_start(out=dbg_hagg0[:], in_=hagg[:])
                    nc.sync.dma_start(out=dbg_stat0[:], in_=stat_g[:])
                    nc.sync.dma_start(out=dbg_h0[:], in_=h_out[:])

            # ---- global mean pool + MLP readout ----
            h_fin = hbufs[L % 2]
            ppool = pmm.tile([G, D], f32, tag="pmlp")
            for t in range(NT):
                cw = NT_W[t]
                pt = pmm.tile([128, 128], f32, tag="pm")
                nc.tensor.matmul(out=pt[:cw, :], lhsT=h_fin[:, t * 128:t * 128 + cw],
                                 rhs=ident_sb[:], start=True, stop=True)
                hr = wk.tile([128, 128], f32, tag="hr")
                nc.vector.tensor_copy(out=hr[:cw, :], in_=pt[:cw, :])
                Pm = stp.tile([128, G], f32, tag="Pm")
                nc.vector.tensor_scalar(out=Pm[:], in0=iota_sb[:, :G],
                                        scalar1=bl_sb[:, t:t + 1], scalar2=None,
                                        op0=Alu.is_equal)
                nc.tensor.matmul(out=ppool[:], lhsT=Pm[:], rhs=hr[:],
                                 start=(t == 0), stop=(t == NT - 1))
            pool_sb = wk.tile([G, D], f32, tag="pool_sb")
            nc.vector.tensor_copy(out=pool_sb[:], in_=ppool[:])
            nc.sync.dma_start(out=pool_in[:], in_=pool_sb[:])
            nc.gpsimd.collective_compute(
                "AllReduce", Alu.add, replica_groups=rg,
                ins=[pool_in.opt()], outs=[pool_out.opt()],
            )
            pg = wk.tile([G, D], f32, tag="pg")
            nc.sync.dma_start(out=pg[:], in_=pool_out[:])
            hg = wk.tile([G, D], f32, tag="hg")
            nc.vector.tensor_scalar(out=hg[:], in0=pg[:], scalar1=invc_sb[:, 0:1],
                                    scalar2=None, op0=Alu.mult)
            # hg_T = hg.T  [D, G]
            pt2 = pmm.tile([128, G], f32, tag="pmlp")
            nc.tensor.matmul(out=pt2[:], lhsT=hg[:], rhs=ident_sb[:G, :G],
                             start=True, stop=True)
            hgT = wk.tile([128, G], f32, tag="hgT")
            nc.vector.tensor_copy(out=hgT[:], in_=pt2[:])
            # z1 = relu(W_r1.T @ hgT + b_r1)
            pz1 = pmm.tile([D // 2, G], f32, tag="pmlp")
            nc.tensor.matmul(out=pz1[:], lhsT=wr1_sb[:], rhs=hgT[:],
                             start=True, stop=True)
            z1 = wk.tile([D // 2, G], f32, tag="z1")
            nc.scalar.activation(out=z1[:], in_=pz1[:], func=Act.Relu,
                                 bias=br1_sb[:, 0:1], scale=1.0)
            pz2 = pmm.tile([D // 4, G], f32, tag="pmlp")
            nc.tensor.matmul(out=pz2[:], lhsT=wr2_sb[:], rhs=z1[:],
                             start=True, stop=True)
            z2 = wk.tile([D // 4, G], f32, tag="z2")
            nc.scalar.activation(out=z2[:], in_=pz2[:], func=Act.Relu,
                                 bias=br2_sb[:, 0:1], scale=1.0)
            pz3 = pmm.tile([NC, G], f32, tag="pmlp")
            nc.tensor.matmul(out=pz3[:], lhsT=wr3_sb[:], rhs=z2[:],
                             start=True, stop=True)
            z3 = wk.tile([NC, G], f32, tag="z3")
            nc.scalar.activation(out=z3[:], in_=pz3[:], func=Act.Identity,
                                 bias=br3_sb[:, 0:1], scale=1.0)
            nc.sync.dma_start(out=out_p[:], in_=z3[:])

    nc.compile()
    return nc


_CACHE = {}


def kernel(x, edge_index, batch, W_emb, b_emb, W_gcn, b_gcn,
           bn_gamma, bn_beta, W_r1, b_r1, W_r2, b_r2, W_r3, b_r3):
    import os
    from concourse.bass_utils import run_bass_kernel_spmd
    trace = bool(os.environ.get("BASS_KERNEL_TRACE"))

    x = np.asarray(x, dtype=np.float32)
    meta, per_core, inv_cnt = _preprocess(x, edge_index, batch)

    import os
    key = (meta["TILES"], meta["IDXCOLS"], tuple(map(tuple, meta["T"])),
           bool(os.environ.get("BASS_KERNEL_DEBUG")))
    if key not in _CACHE:
        _CACHE[key] = _build(meta)
    nc = _CACHE[key]

    iota = np.tile(np.arange(WIN, dtype=np.float32), (128, 1))
    ident = np.eye(128, dtype=np.float32)
    shared = dict(
        W_emb=np.asarray(W_emb, np.float32),
        b_emb=np.asarray(b_emb, np.float32).reshape(D, 1),
        W_gcn=np.asarray(W_gcn, np.float32),
        bgcn_t=np.ascontiguousarray(np.asarray(b_gcn, np.float32).T),
        gamma_t=np.ascontiguousarray(np.asarray(bn_gamma, np.float32).T),
        beta_t=np.ascontiguousarray(np.asarray(bn_beta, np.float32).T),
        W_r1=np.asarray(W_r1, np.float32),
        b_r1=np.asarray(b_r1, np.float32).reshape(D // 2, 1),
        W_r2=np.asarray(W_r2, np.float32),
        b_r2=np.asarray(b_r2, np.float32).reshape(D // 4, 1),
        W_r3=np.asarray(W_r3, np.float32),
        b_r3=np.asarray(b_r3, np.float32).reshape(NC, 1),
        iota=iota, ident=ident, inv_cnt=inv_cnt,
    )
    in_maps = []
    for c in range(C):
        m = dict(shared)
        m["x_t"] = per_core[c]["x_t"]
        m["idx"] = per_core[c]["idx"]
        m["dstl"] = per_core[c]["dstl"]
        m["norm"] = per_core[c]["norm"]
        m["ns"] = per_core[c]["ns"]
        m["bl"] = per_core[c]["bl"]
        in_maps.append(m)

    res = run_bass_kernel_spmd(nc, in_maps, core_ids=list(range(C)), trace=trace)
    if trace and res.exec_time_ns is not None:
        print(f"HW exec time: {res.exec_time_ns} ns")
    out = res.results[0]["out"]  # [NC, G]
    return np.ascontiguousarray(out.T.astype(np.float32))  # [G, NC]



# revision 8
# speedup vs baseline: 1.6370x; 1.6370x over previous
"""GCNNet2 on 8 Trainium2 NeuronCores (Bass/Tile).

Strategy: shard nodes (contiguous 6250-node ranges) across 8 cores; each core
owns the aggregation for its dst range. The normalized adjacency is graph-
static, so the one-hot scatter matrices S (with the dst-side degree factor
folded in) are precomputed on host in bf16 and streamed from DRAM each layer.
m rows are pre-scaled by the src-side degree factor on-chip, so gathered rows
times S gives exactly norm-weighted messages; the b_gcn bias cancels inside
BatchNorm and is dropped. Per layer: m = h @ W (bf16) scaled by dinv, written
to two bounce halves that AllGather separately (half-A aggregation overlaps
the half-B collective); big dma_gather calls (up to 48 edge tiles each) fetch
m[src] rows; PE accumulates seed (self-loop diag) + edge one-hot matmuls per
128-dst window in PSUM; BatchNorm via a tiny AllReduce of per-core sums;
fused scale/bias/relu + residual. Global mean pool via indicator matmul +
AllReduce, then the MLP readout (replicated).
"""
import numpy as np

# Problem constants (hardcoded per contract; kernel.py must be self-contained)
N = 50000
E = 800000
DIN = 146
D = 128
G = 64
L = 4
NC = 10
EPS = 1e-5

C = 8          # cores
NL = N // C    # 6250 nodes per core
NT = (NL + 127) // 128                # 49 node tiles (= aggregation windows)
NT_W = [min(128, NL - t * 128) for t in range(NT)]
HLOC = 3200    # local-node split: tiles 0-24 -> half A, 25-48 -> half B
TA = HLOC // 128          # 25 tiles in half A
HA = C * HLOC             # 25600 rows in table A (int16-safe)
HB = C * (NL - HLOC)      # 24400 rows in table B
GRP = 4        # windows per gather group
MAXJ = 8      # max edge tiles per dma_gather call


def _static_structure(counts):
    """counts: [C, NT, 2] per-core edge counts per (window, half).
    Returns core-invariant tile/call structure."""
    T = np.maximum.reduce(-(-counts // 128), axis=0)  # [NT, 2] ceil/max over cores
    groups = [list(range(g, min(g + GRP, NT))) for g in range(0, NT, GRP)]
    tile_base = {}
    tile_meta = []   # (w, h) per static tile
    calls = []       # (h, t0, j, icol_off) in issue order
    icol = 0
    for ws in groups:
        for h in (0, 1):
            t0g = len(tile_meta)
            for w in ws:
                tile_base[(w, h)] = len(tile_meta)
                tile_meta.extend([(w, h)] * int(T[w, h]))
            ntg = len(tile_meta) - t0g
            r = 0
            while r < ntg:
                j = min(MAXJ, ntg - r)
                calls.append((h, t0g + r, j, icol))
                icol += 8 * j
                r += j
    TILES = len(tile_meta)
    # map: static tile -> (call index, slot within call)
    tile_call = {}
    for ci, (h, t0, j, off) in enumerate(calls):
        for jj in range(j):
            tile_call[t0 + jj] = (ci, jj)
    return dict(T=T, groups=groups, tile_base=tile_base, tile_meta=tile_meta,
                calls=calls, TILES=TILES, IDXCOLS=icol, tile_call=tile_call)


def _preprocess(x, edge_index, batch):
    src = np.asarray(edge_index[0], dtype=np.int64)
    dst = np.asarray(edge_index[1], dtype=np.int64)
    batch = np.asarray(batch, dtype=np.int64)

    deg = (np.bincount(dst, minlength=N) + 1).astype(np.float32)  # + self-loop
    dinv = (1.0 / np.sqrt(deg)).astype(np.float32)

    # gather-table index (two tables split by owner-local offset)
    oc = src // NL
    osl = src % NL
    half = (osl >= HLOC).astype(np.int64)
    idx16 = np.where(half == 0, oc * HLOC + osl,
                     oc * (NL - HLOC) + (osl - HLOC)).astype(np.int16)

    core = dst // NL
    w = (dst % NL) // 128
    dstl = (dst % NL) % 128

    key = (core * NT + w) * 2 + half
    order = np.argsort(key, kind="stable")
    key_s = key[order]
    bounds = np.searchsorted(key_s, np.arange(C * NT * 2 + 1))
    counts = np.zeros((C, NT, 2), dtype=np.int64)
    for c in range(C):
        for ww in range(NT):
            for h in range(2):
                k = (c * NT + ww) * 2 + h
                counts[c, ww, h] = bounds[k + 1] - bounds[k]

    meta = _static_structure(counts)
    T, TILES, IDXCOLS = meta["T"], meta["TILES"], meta["IDXCOLS"]
    tile_base, calls = meta["tile_base"], meta["calls"]

    try:
        import ml_dtypes
        bf16 = ml_dtypes.bfloat16
    except ImportError:  # pragma: no cover
        from jax import numpy as jnp
        bf16 = jnp.bfloat16

    per_core = []
    for c in range(C):
        S = np.zeros((128, TILES * 128), dtype=np.float32)
        flat_idx = np.zeros((TILES, 128), dtype=np.int16)
        for ww in range(NT):
            for h in range(2):
                k = (c * NT + ww) * 2 + h
                el = order[bounds[k]:bounds[k + 1]]
                if len(el) == 0:
                    continue
                tb = tile_base[(ww, h)]
                s = np.arange(len(el))
                ti = tb + s // 128
                slot = s % 128
                S[slot, ti * 128 + dstl[el]] = dinv[dst[el]]
                flat_idx[ti, slot] = idx16[el]
        # pack gather indices per call: idx i of call -> [i%16, off + i//16]
        idx_arr = np.zeros((16, IDXCOLS), dtype=np.int16)
        for (h, t0, j, off) in calls:
            seq = flat_idx[t0:t0 + j].reshape(-1)
            idx_arr[:, off:off + 8 * j] = seq.reshape(8 * j, 16).T
        idx_rep = np.tile(idx_arr, (8, 1))

        lo = c * NL
        dinv_l = dinv[lo:lo + NL]
        dinv_p = np.zeros((128, NT), dtype=np.float32)
        sd = np.zeros((128, NT * 128), dtype=np.float32)
        Pm = np.zeros((128, NT * G), dtype=np.float32)
        for t in range(NT):
            cw = NT_W[t]
            dinv_p[:cw, t] = dinv_l[t * 128:t * 128 + cw]
            q = np.arange(cw)
            sd[q, t * 128 + q] = dinv_l[t * 128 + q]
            Pm[q, t * G + batch[lo + t * 128 + q]] = 1.0

        x_c = np.asarray(x[lo:lo + NL], dtype=np.float32).T  # [DIN, NL]
        per_core.append(dict(
            idx=idx_rep,
            S=np.ascontiguousarray(S).astype(bf16),
            sd=sd.astype(bf16),
            Pm=Pm.astype(bf16),
            dinv_p=dinv_p,
            x1_t=np.ascontiguousarray(x_c[:128]).astype(bf16),
            x2_t=np.ascontiguousarray(x_c[128:]).astype(bf16),
        ))

    cnt = np.bincount(batch, minlength=G).astype(np.float32)
    inv_cnt = (1.0 / np.maximum(cnt, 1.0)).astype(np.float32).reshape(G, 1)
    return meta, per_core, inv_cnt, bf16


def _build(meta):
    import concourse.bacc as bacc
    import concourse.mybir as mybir
    import concourse.tile as tile

    f32 = mybir.dt.float32
    bf = mybir.dt.bfloat16
    i16 = mybir.dt.int16
    Alu = mybir.AluOpType
    Act = mybir.ActivationFunctionType
    Axis = mybir.AxisListType

    TILES = meta["TILES"]
    IDXCOLS = meta["IDXCOLS"]
    tile_meta = meta["tile_meta"]
    calls = meta["calls"]
    groups = meta["groups"]
    tile_base = meta["tile_base"]
    T = meta["T"]
    tile_call = meta["tile_call"]

    nc = bacc.Bacc(None, target_bir_lowering=False, num_swdge_queues=4)

    P = {}
    P["x1_t"] = nc.declare_dram_parameter("x1_t", [128, NL], bf, isOutput=False)
    P["x2_t"] = nc.declare_dram_parameter("x2_t", [DIN - 128, NL], bf, isOutput=False)
    P["idx"] = nc.declare_dram_parameter("idx", [128, IDXCOLS], i16, isOutput=False)
    P["S"] = nc.declare_dram_parameter("S", [128, TILES * 128], bf, isOutput=False)
    P["sd"] = nc.declare_dram_parameter("sd", [128, NT * 128], bf, isOutput=False)
    P["Pm"] = nc.declare_dram_parameter("Pm", [128, NT * G], bf, isOutput=False)
    P["dinv_p"] = nc.declare_dram_parameter("dinv_p", [128, NT], f32, isOutput=False)
    P["wemb1"] = nc.declare_dram_parameter("wemb1", [128, D], bf, isOutput=False)
    P["wemb2"] = nc.declare_dram_parameter("wemb2", [DIN - 128, D], bf, isOutput=False)
    P["b_emb"] = nc.declare_dram_parameter("b_emb", [D, 1], f32, isOutput=False)
    P["W_gcn"] = nc.declare_dram_parameter("W_gcn", [L, D, D], bf, isOutput=False)
    P["gamma_t"] = nc.declare_dram_parameter("gamma_t", [D, L], f32, isOutput=False)
    P["beta_t"] = nc.declare_dram_parameter("beta_t", [D, L], f32, isOutput=False)
    P["W_r1"] = nc.declare_dram_parameter("W_r1", [D, D // 2], f32, isOutput=False)
    P["b_r1"] = nc.declare_dram_parameter("b_r1", [D // 2, 1], f32, isOutput=False)
    P["W_r2"] = nc.declare_dram_parameter("W_r2", [D // 2, D // 4], f32, isOutput=False)
    P["b_r2"] = nc.declare_dram_parameter("b_r2", [D // 4, 1], f32, isOutput=False)
    P["W_r3"] = nc.declare_dram_parameter("W_r3", [D // 4, NC], f32, isOutput=False)
    P["b_r3"] = nc.declare_dram_parameter("b_r3", [NC, 1], f32, isOutput=False)
    P["ident"] = nc.declare_dram_parameter("ident", [128, 128], f32, isOutput=False)
    P["identb"] = nc.declare_dram_parameter("identb", [128, 128], bf, isOutput=False)
    P["inv_cnt"] = nc.declare_dram_parameter("inv_cnt", [G, 1], f32, isOutput=False)
    out_p = nc.declare_dram_parameter("out", [NC, G], f32, isOutput=True)

    rg = [list(range(C))]

    with tile.TileContext(nc) as tc:
        with (
            tc.tile_pool(name="const", bufs=1) as cst,
            tc.tile_pool(name="hbuf", bufs=1) as hbuf,
            tc.tile_pool(name="gd", bufs=3) as gd,
            tc.tile_pool(name="sp", bufs=3) as sp,
            tc.tile_pool(name="work", bufs=3) as wk,
            tc.tile_pool(name="xst", bufs=3) as xst,
            tc.tile_pool(name="pag", bufs=4, space="PSUM") as pag,
            tc.tile_pool(name="pmm", bufs=2, space="PSUM") as pmm,
            tc.tile_pool(name="pmo", bufs=1, space="PSUM") as pmo,
            tc.tile_pool(name="dram", bufs=1, space="DRAM") as drp,
        ):
            def load_const(name, shape, dt=f32):
                t = cst.tile(shape, dt, tag=f"c_{name}")
                nc.sync.dma_start(out=t[:], in_=P[name][:])
                return t

            idx_sb = load_const("idx", [128, IDXCOLS], i16)
            sd_sb = load_const("sd", [128, NT * 128], bf)
            pm_sb = load_const("Pm", [128, NT * G], bf)
            dinv_sb = load_const("dinv_p", [128, NT])
            wemb1 = load_const("wemb1", [128, D], bf)
            wemb2 = load_const("wemb2", [DIN - 128, D], bf)
            bemb_sb = load_const("b_emb", [D, 1])
            gamma_sb = load_const("gamma_t", [D, L])
            beta_sb = load_const("beta_t", [D, L])
            wr1_sb = load_const("W_r1", [D, D // 2])
            br1_sb = load_const("b_r1", [D // 2, 1])
            wr2_sb = load_const("W_r2", [D // 2, D // 4])
            br2_sb = load_const("b_r2", [D // 4, 1])
            wr3_sb = load_const("W_r3", [D // 4, NC])
            br3_sb = load_const("b_r3", [NC, 1])
            ident_sb = load_const("ident", [128, 128])
            identb_sb = load_const("identb", [128, 128], bf)
            invc_sb = load_const("inv_cnt", [G, 1])
            wgcn_sb = cst.tile([128, L * D], bf)
            for l in range(L):
                nc.sync.dma_start(out=wgcn_sb[:, l * D:(l + 1) * D],
                                  in_=P["W_gcn"][l])

            hA = hbuf.tile([128, NT * 128], bf)
            hB = hbuf.tile([128, NT * 128], bf)
            hagg = hbuf.tile([128, NT * 128], f32)
            m_sb = hbuf.tile([128, NT * 128], bf)
            sums = hbuf.tile([128, NT], f32)
            sumsq = hbuf.tile([128, NT], f32)

            m_bounceA = drp.tile([HLOC, D], bf)
            m_bounceB = drp.tile([NL - HLOC, D], bf)
            m_fullA = [drp.tile([HA, D], bf, name=f"m_fullA{l}")
                       for l in range(L)]
            m_fullB = [drp.tile([HB, D], bf, name=f"m_fullB{l}")
                       for l in range(L)]
            stat_in = drp.tile([128, 2], f32)
            stat_out = [drp.tile([128, 2], f32, name=f"stat_out{l}")
                        for l in range(L)]
            pool_in = drp.tile([G, D], f32)
            pool_out = drp.tile([G, D], f32)

            # ---- embedding: h0_T = W_emb.T @ x_T + b_emb ----
            for t in range(NT):
                c0 = t * 128
                cw = NT_W[t]
                x1 = xst.tile([128, 128], bf, tag="x1")
                nc.sync.dma_start(out=x1[:, :cw], in_=P["x1_t"][:, c0:c0 + cw])
                x2 = xst.tile([DIN - 128, 128], bf, tag="x2")
                nc.sync.dma_start(out=x2[:, :cw], in_=P["x2_t"][:, c0:c0 + cw])
                pe = pmm.tile([128, 128], f32, tag="pm")
                nc.tensor.matmul(out=pe[:, :cw], lhsT=wemb1[:], rhs=x1[:, :cw],
                                 start=True, stop=False)
                nc.tensor.matmul(out=pe[:, :cw], lhsT=wemb2[:], rhs=x2[:, :cw],
                                 start=False, stop=True)
                nc.scalar.activation(out=hA[:, c0:c0 + cw], in_=pe[:, :cw],
                                     func=Act.Identity, bias=bemb_sb[:, 0:1],
                                     scale=1.0)

            hbufs = [hA, hB]
            qn = [0]

            # ---- GCN layers ----
            for l in range(L):
                h_in = hbufs[l % 2]
                h_out = hbufs[(l + 1) % 2]
                W_l = wgcn_sb[:, l * D:(l + 1) * D]

                # m = dinv * (h @ W): node-major bf16 tiles, kept in SBUF and
                # bounced to DRAM halves for the split AllGather
                for t in range(NT):
                    cw = NT_W[t]
                    pm = pmm.tile([128, D], f32, tag="pm")
                    nc.tensor.matmul(out=pm[:cw, :],
                                     lhsT=h_in[:, t * 128:t * 128 + cw],
                                     rhs=W_l, start=True, stop=True)
                    nc.scalar.activation(out=m_sb[:cw, t * D:(t + 1) * D],
                                         in_=pm[:cw, :], func=Act.Identity,
                                         bias=0.0, scale=dinv_sb[:cw, t:t + 1])
                    if t < TA:
                        nc.sync.dma_start(
                            out=m_bounceA[t * 128:t * 128 + cw, :],
                            in_=m_sb[:cw, t * D:(t + 1) * D])
                    else:
                        r0 = (t - TA) * 128
                        nc.sync.dma_start(
                            out=m_bounceB[r0:r0 + cw, :],
                            in_=m_sb[:cw, t * D:(t + 1) * D])
                    if t == TA - 1:
                        nc.gpsimd.collective_compute(
                            "AllGather", Alu.bypass, replica_groups=rg,
                            ins=[m_bounceA.opt()], outs=[m_fullA[l].opt()])
                nc.gpsimd.collective_compute(
                    "AllGather", Alu.bypass, replica_groups=rg,
                    ins=[m_bounceB.opt()], outs=[m_fullB[l].opt()])

                # issue all gathers + S streams (pipelined via pool bufs)
                gts = {}
                sgs = {}
                for ci, (h, t0, j, off) in enumerate(calls):
                    gt = gd.tile([128, MAXJ, D], bf, tag="gt")
                    tab = m_fullA[l] if h == 0 else m_fullB[l]
                    nc.gpsimd.dma_gather(
                        gt[:, :j, :], tab[:], idx_sb[:, off:off + 8 * j],
                        128 * j, 128 * j, D, queue_num=qn[0] % 4)
                    qn[0] += 1
                    sg = sp.tile([128, MAXJ * D], bf, tag="sg")
                    nc.sync.dma_start(out=sg[:, :j * D],
                                      in_=P["S"][:, t0 * D:(t0 + j) * D])
                    for jj in range(j):
                        gts[t0 + jj] = (gt, jj)
                        sgs[t0 + jj] = (sg, jj)

                # aggregate per window: seed opens PSUM, edge tiles accumulate
                for ws in groups:
                    for w in ws:
                        cw = NT_W[w]
                        tiles_w = (list(range(tile_base[(w, 0)],
                                              tile_base[(w, 0)] + int(T[w, 0])))
                                   + list(range(tile_base[(w, 1)],
                                                tile_base[(w, 1)] + int(T[w, 1]))))
                        pw = pag.tile([128, 128], f32, tag="pw")
                        nc.tensor.matmul(
                            out=pw[:], lhsT=m_sb[:cw, w * D:(w + 1) * D],
                            rhs=sd_sb[:cw, w * 128:(w + 1) * 128],
                            start=True, stop=(len(tiles_w) == 0))
                        for i, ti in enumerate(tiles_w):
                            gt, jj = gts[ti]
                            sg, js = sgs[ti]
                            nc.tensor.matmul(
                                out=pw[:], lhsT=gt[:, jj, :],
                                rhs=sg[:, js * D:(js + 1) * D],
                                start=False, stop=(i == len(tiles_w) - 1))
                        w0 = w * 128
                        nc.vector.tensor_copy(out=hagg[:, w0:w0 + cw],
                                              in_=pw[:, :cw])
                        nc.vector.reduce_sum(out=sums[:, w:w + 1],
                                             in_=hagg[:, w0:w0 + cw], axis=Axis.X)
                        sq = wk.tile([128, 128], f32, tag="sq")
                        nc.scalar.square(out=sq[:, :cw], in_=hagg[:, w0:w0 + cw])
                        nc.vector.reduce_sum(out=sumsq[:, w:w + 1],
                                             in_=sq[:, :cw], axis=Axis.X)

                # global BN stats (b_gcn cancels inside BN and is dropped)
                stat_sb = wk.tile([128, 2], f32, tag="stat")
                nc.vector.reduce_sum(out=stat_sb[:, 0:1], in_=sums[:], axis=Axis.X)
                nc.vector.reduce_sum(out=stat_sb[:, 1:2], in_=sumsq[:], axis=Axis.X)
                nc.sync.dma_start(out=stat_in[:], in_=stat_sb[:])
                nc.gpsimd.collective_compute(
                    "AllReduce", Alu.add, replica_groups=rg,
                    ins=[stat_in.opt()], outs=[stat_out[l].opt()])
                stat_g = wk.tile([128, 2], f32, tag="statg")
                nc.sync.dma_start(out=stat_g[:], in_=stat_out[l][:])

                mu = wk.tile([128, 1], f32, tag="mu")
                nc.vector.tensor_scalar(out=mu[:], in0=stat_g[:, 0:1],
                                        scalar1=1.0 / N, scalar2=None, op0=Alu.mult)
                ex2 = wk.tile([128, 1], f32, tag="ex2")
                nc.vector.tensor_scalar(out=ex2[:], in0=stat_g[:, 1:2],
                                        scalar1=1.0 / N, scalar2=None, op0=Alu.mult)
                musq = wk.tile([128, 1], f32, tag="musq")
                nc.vector.tensor_tensor(out=musq[:], in0=mu[:], in1=mu[:], op=Alu.mult)
                var = wk.tile([128, 1], f32, tag="var")
                nc.vector.tensor_tensor(out=var[:], in0=ex2[:], in1=musq[:],
                                        op=Alu.subtract)
                var2 = wk.tile([128, 1], f32, tag="var2")
                nc.vector.tensor_scalar(out=var2[:], in0=var[:], scalar1=float(EPS),
                                        scalar2=None, op0=Alu.add)
                stdv = wk.tile([128, 1], f32, tag="stdv")
                nc.scalar.activation(out=stdv[:], in_=var2[:], func=Act.Sqrt)
                rinv = wk.tile([128, 1], f32, tag="rinv")
                nc.vector.reciprocal(out=rinv[:], in_=stdv[:])
                a_t = wk.tile([128, 1], f32, tag="a_t")
                nc.vector.tensor_tensor(out=a_t[:], in0=gamma_sb[:, l:l + 1],
                                        in1=rinv[:], op=Alu.mult)
                t1 = wk.tile([128, 1], f32, tag="t1")
                nc.vector.tensor_tensor(out=t1[:], in0=mu[:], in1=a_t[:], op=Alu.mult)
                b2 = wk.tile([128, 1], f32, tag="b2")
                nc.vector.tensor_tensor(out=b2[:], in0=beta_sb[:, l:l + 1],
                                        in1=t1[:], op=Alu.subtract)

                for w in range(NT):
                    cw = NT_W[w]
                    w0 = w * 128
                    rl = wk.tile([128, 128], bf, tag="rl")
                    nc.scalar.activation(out=rl[:, :cw], in_=hagg[:, w0:w0 + cw],
                                         func=Act.Relu, bias=b2[:, 0:1],
                                         scale=a_t[:, 0:1])
                    nc.vector.tensor_add(out=h_out[:, w0:w0 + cw],
                                         in0=rl[:, :cw], in1=h_in[:, w0:w0 + cw])

            # ---- global mean pool + MLP readout ----
            h_fin = hbufs[L % 2]
            ppool = pmo.tile([G, D], f32, tag="ppool")
            for t in range(NT):
                cw = NT_W[t]
                pt = pmm.tile([128, 128], f32, tag="pm")
                nc.tensor.matmul(out=pt[:cw, :],
                                 lhsT=h_fin[:, t * 128:t * 128 + cw],
                                 rhs=identb_sb[:], start=True, stop=True)
                hr = wk.tile([128, 128], bf, tag="hr")
                nc.scalar.activation(out=hr[:cw, :], in_=pt[:cw, :],
                                     func=Act.Identity, bias=0.0, scale=1.0)
                nc.tensor.matmul(out=ppool[:], lhsT=pm_sb[:cw, t * G:(t + 1) * G],
                                 rhs=hr[:cw, :],
                                 start=(t == 0), stop=(t == NT - 1))
            pool_sb = wk.tile([G, D], f32, tag="pool_sb")
            nc.vector.tensor_copy(out=pool_sb[:], in_=ppool[:])
            nc.sync.dma_start(out=pool_in[:], in_=pool_sb[:])
            nc.gpsimd.collective_compute(
                "AllReduce", Alu.add, replica_groups=rg,
                ins=[pool_in.opt()], outs=[pool_out.opt()])
            pg = wk.tile([G, D], f32, tag="pg")
            nc.sync.dma_start(out=pg[:], in_=pool_out[:])
            hg = wk.tile([G, D], f32, tag="hg")
            nc.vector.tensor_scalar(out=hg[:], in0=pg[:], scalar1=invc_sb[:, 0:1],
                                    scalar2=None, op0=Alu.mult)
            pt2 = pmo.tile([128, G], f32, tag="pmlp")
            nc.tensor.matmul(out=pt2[:], lhsT=hg[:], rhs=ident_sb[:G, :G],
                             start=True, stop=True)
            hgT = wk.tile([128, G], f32, tag="hgT")
            nc.vector.tensor_copy(out=hgT[:], in_=pt2[:])
            pz1 = pmo.tile([D // 2, G], f32, tag="pmlp")
            nc.tensor.matmul(out=pz1[:], lhsT=wr1_sb[:], rhs=hgT[:],
                             start=True, stop=True)
            z1 = wk.tile([D // 2, G], f32, tag="z1")
            nc.scalar.activation(out=z1[:], in_=pz1[:], func=Act.Relu,
                                 bias=br1_sb[:, 0:1], scale=1.0)
            pz2 = pmo.tile([D // 4, G], f32, tag="pmlp")
            nc.tensor.matmul(out=pz2[:], lhsT=wr2_sb[:], rhs=z1[:],
                             start=True, stop=True)
            z2 = wk.tile([D // 4, G], f32, tag="z2")
            nc.scalar.activation(out=z2[:], in_=pz2[:], func=Act.Relu,
                                 bias=br2_sb[:, 0:1], scale=1.0)
            pz3 = pmo.tile([NC, G], f32, tag="pmlp")
            nc.tensor.matmul(out=pz3[:], lhsT=wr3_sb[:], rhs=z2[:],
                             start=True, stop=True)
            z3 = wk.tile([NC, G], f32, tag="z3")
            nc.scalar.activation(out=z3[:], in_=pz3[:], func=Act.Identity,
                                 bias=br3_sb[:, 0:1], scale=1.0)
            nc.sync.dma_start(out=out_p[:], in_=z3[:])

    nc.compile()
    return nc


_CACHE = {}


def kernel(x, edge_index, batch, W_emb, b_emb, W_gcn, b_gcn,
           bn_gamma, bn_beta, W_r1, b_r1, W_r2, b_r2, W_r3, b_r3):
    import os
    from concourse.bass_utils import run_bass_kernel_spmd
    trace = bool(os.environ.get("BASS_KERNEL_TRACE"))

    x = np.asarray(x, dtype=np.float32)
    meta, per_core, inv_cnt, bf16 = _preprocess(x, edge_index, batch)

    key = (meta["TILES"], meta["IDXCOLS"], tuple(map(tuple, meta["T"])))
    if key not in _CACHE:
        _CACHE[key] = _build(meta)
    nc = _CACHE[key]

    W_emb = np.asarray(W_emb, np.float32)
    shared = dict(
        wemb1=np.ascontiguousarray(W_emb[:128]).astype(bf16),
        wemb2=np.ascontiguousarray(W_emb[128:]).astype(bf16),
        b_emb=np.asarray(b_emb, np.float32).reshape(D, 1),
        W_gcn=np.asarray(W_gcn, np.float32).astype(bf16),
        gamma_t=np.ascontiguousarray(np.asarray(bn_gamma, np.float32).T),
        beta_t=np.ascontiguousarray(np.asarray(bn_beta, np.float32).T),
        W_r1=np.asarray(W_r1, np.float32),
        b_r1=np.asarray(b_r1, np.float32).reshape(D // 2, 1),
        W_r2=np.asarray(W_r2, np.float32),
        b_r2=np.asarray(b_r2, np.float32).reshape(D // 4, 1),
        W_r3=np.asarray(W_r3, np.float32),
        b_r3=np.asarray(b_r3, np.float32).reshape(NC, 1),
        ident=np.eye(128, dtype=np.float32),
        identb=np.eye(128, dtype=np.float32).astype(bf16),
        inv_cnt=inv_cnt,
    )
    in_maps = []
    for c in range(C):
        m = dict(shared)
        m.update(per_core[c])
        in_maps.append(m)

    res = run_bass_kernel_spmd(nc, in_maps, core_ids=list(range(C)), trace=trace)
    if trace and res.exec_time_ns is not None:
        print(f"HW exec time: {res.exec_time_ns} ns")
    out = res.results[0]["out"]  # [NC, G]
    return np.ascontiguousarray(out.T.astype(np.float32))  # [G, NC]


# revision 15
# speedup vs baseline: 2.1823x; 1.3331x over previous
"""GCNNet2 on 8 Trainium2 NeuronCores (Bass/Tile).

Strategy: shard nodes (contiguous 6250-node ranges) across 8 cores; each core
owns the aggregation for its dst range. The normalized adjacency is graph-
static, so the one-hot scatter matrices S (with the dst-side degree factor
folded in) are precomputed on host in bf16 and streamed from DRAM each layer.
m rows are pre-scaled by the src-side degree factor on-chip, so gathered rows
times S gives exactly norm-weighted messages; the b_gcn bias cancels inside
BatchNorm and is dropped. Per layer: m = h @ W (bf16) scaled by dinv, written
to two bounce halves that AllGather separately (half-A aggregation overlaps
the half-B collective); big dma_gather calls (up to 48 edge tiles each) fetch
m[src] rows; PE accumulates seed (self-loop diag) + edge one-hot matmuls per
128-dst window in PSUM; BatchNorm via a tiny AllReduce of per-core sums;
fused scale/bias/relu + residual. Global mean pool via indicator matmul +
AllReduce, then the MLP readout (replicated).
"""
import numpy as np

# Problem constants (hardcoded per contract; kernel.py must be self-contained)
N = 50000
E = 800000
DIN = 146
D = 128
G = 64
L = 4
NC = 10
EPS = 1e-5

C = 8          # cores
NL = N // C    # 6250 nodes per core
NT = (NL + 127) // 128                # 49 node tiles (= aggregation windows)
NT_W = [min(128, NL - t * 128) for t in range(NT)]
HLOC = 3200    # local-node split: tiles 0-24 -> half A, 25-48 -> half B
TA = HLOC // 128          # 25 tiles in half A
HA = C * HLOC             # 25600 rows in table A (int16-safe)
HB = C * (NL - HLOC)      # 24400 rows in table B
GRP = 4        # windows per gather group
MAXJ = 8      # max edge tiles per dma_gather call (1024 idxs; >1024 hangs)


def _static_structure(counts):
    """counts: [C, NT, 2] per-core edge counts per (window, half).
    Returns core-invariant tile/call structure."""
    T = np.maximum.reduce(-(-counts // 128), axis=0)  # [NT, 2] ceil/max over cores
    groups = [list(range(g, min(g + GRP, NT))) for g in range(0, NT, GRP)]
    tile_base = {}
    tile_meta = []   # (w, h) per static tile
    calls = []       # (h, t0, j, icol_off) in issue order
    icol = 0
    for ws in groups:
        for h in (0, 1):
            t0g = len(tile_meta)
            for w in ws:
                tile_base[(w, h)] = len(tile_meta)
                tile_meta.extend([(w, h)] * int(T[w, h]))
            ntg = len(tile_meta) - t0g
            r = 0
            while r < ntg:
                j = min(MAXJ, ntg - r)
                calls.append((h, t0g + r, j, icol))
                icol += 8 * j
                r += j
    TILES = len(tile_meta)
    # map: static tile -> (call index, slot within call)
    tile_call = {}
    for ci, (h, t0, j, off) in enumerate(calls):
        for jj in range(j):
            tile_call[t0 + jj] = (ci, jj)
    return dict(T=T, groups=groups, tile_base=tile_base, tile_meta=tile_meta,
                calls=calls, TILES=TILES, IDXCOLS=icol, tile_call=tile_call)


def _preprocess(x, edge_index, batch):
    src = np.asarray(edge_index[0], dtype=np.int64)
    dst = np.asarray(edge_index[1], dtype=np.int64)
    batch = np.asarray(batch, dtype=np.int64)

    deg = (np.bincount(dst, minlength=N) + 1).astype(np.float32)  # + self-loop
    dinv = (1.0 / np.sqrt(deg)).astype(np.float32)

    # gather-table index (two tables split by owner-local offset)
    oc = src // NL
    osl = src % NL
    half = (osl >= HLOC).astype(np.int64)
    idx16 = np.where(half == 0, oc * HLOC + osl,
                     oc * (NL - HLOC) + (osl - HLOC)).astype(np.int16)

    core = dst // NL
    w = (dst % NL) // 128
    dstl = (dst % NL) % 128

    key = (core * NT + w) * 2 + half
    order = np.argsort(key, kind="stable")
    key_s = key[order]
    bounds = np.searchsorted(key_s, np.arange(C * NT * 2 + 1))
    counts = np.zeros((C, NT, 2), dtype=np.int64)
    for c in range(C):
        for ww in range(NT):
            for h in range(2):
                k = (c * NT + ww) * 2 + h
                counts[c, ww, h] = bounds[k + 1] - bounds[k]

    meta = _static_structure(counts)
    T, TILES, IDXCOLS = meta["T"], meta["TILES"], meta["IDXCOLS"]
    tile_base, calls = meta["tile_base"], meta["calls"]

    try:
        import ml_dtypes
        bf16 = ml_dtypes.bfloat16
    except ImportError:  # pragma: no cover
        from jax import numpy as jnp
        bf16 = jnp.bfloat16

    per_core = []
    for c in range(C):
        S = np.zeros((128, TILES * 128), dtype=np.float32)
        flat_idx = np.zeros((TILES, 128), dtype=np.int16)
        for ww in range(NT):
            for h in range(2):
                k = (c * NT + ww) * 2 + h
                el = order[bounds[k]:bounds[k + 1]]
                if len(el) == 0:
                    continue
                tb = tile_base[(ww, h)]
                s = np.arange(len(el))
                ti = tb + s // 128
                slot = s % 128
                S[slot, ti * 128 + dstl[el]] = dinv[dst[el]]
                flat_idx[ti, slot] = idx16[el]
        # pack gather indices per call: idx i of call -> [i%16, off + i//16]
        idx_arr = np.zeros((16, IDXCOLS), dtype=np.int16)
        for (h, t0, j, off) in calls:
            seq = flat_idx[t0:t0 + j].reshape(-1)
            idx_arr[:, off:off + 8 * j] = seq.reshape(8 * j, 16).T
        idx_rep = np.tile(idx_arr, (8, 1))

        lo = c * NL
        dinv_l = dinv[lo:lo + NL]
        dinv_p = np.zeros((128, NT), dtype=np.float32)
        sd = np.zeros((128, NT * 128), dtype=np.float32)
        Pm = np.zeros((128, NT * G), dtype=np.float32)
        for t in range(NT):
            cw = NT_W[t]
            dinv_p[:cw, t] = dinv_l[t * 128:t * 128 + cw]
            q = np.arange(cw)
            sd[q, t * 128 + q] = dinv_l[t * 128 + q]
            Pm[q, t * G + batch[lo + t * 128 + q]] = 1.0

        x_c = np.asarray(x[lo:lo + NL], dtype=np.float32).T  # [DIN, NL]
        per_core.append(dict(
            idx=idx_rep,
            S=np.ascontiguousarray(S).astype(bf16),
            sd=sd.astype(bf16),
            Pm=Pm.astype(bf16),
            dinv_p=dinv_p,
            x1_t=np.ascontiguousarray(x_c[:128]).astype(bf16),
            x2_t=np.ascontiguousarray(x_c[128:]).astype(bf16),
        ))

    cnt = np.bincount(batch, minlength=G).astype(np.float32)
    inv_cnt = (1.0 / np.maximum(cnt, 1.0)).astype(np.float32).reshape(G, 1)
    return meta, per_core, inv_cnt, bf16


def _build(meta):
    import concourse.bacc as bacc
    import concourse.mybir as mybir
    import concourse.tile as tile

    f32 = mybir.dt.float32
    bf = mybir.dt.bfloat16
    i16 = mybir.dt.int16
    Alu = mybir.AluOpType
    Act = mybir.ActivationFunctionType
    Axis = mybir.AxisListType

    TILES = meta["TILES"]
    IDXCOLS = meta["IDXCOLS"]
    tile_meta = meta["tile_meta"]
    calls = meta["calls"]
    groups = meta["groups"]
    tile_base = meta["tile_base"]
    T = meta["T"]
    tile_call = meta["tile_call"]

    nc = bacc.Bacc(None, target_bir_lowering=False, num_swdge_queues=4)

    P = {}
    P["x1_t"] = nc.declare_dram_parameter("x1_t", [128, NL], bf, isOutput=False)
    P["x2_t"] = nc.declare_dram_parameter("x2_t", [DIN - 128, NL], bf, isOutput=False)
    P["idx"] = nc.declare_dram_parameter("idx", [128, IDXCOLS], i16, isOutput=False)
    P["S"] = nc.declare_dram_parameter("S", [128, TILES * 128], bf, isOutput=False)
    P["sd"] = nc.declare_dram_parameter("sd", [128, NT * 128], bf, isOutput=False)
    P["Pm"] = nc.declare_dram_parameter("Pm", [128, NT * G], bf, isOutput=False)
    P["dinv_p"] = nc.declare_dram_parameter("dinv_p", [128, NT], f32, isOutput=False)
    P["wemb1"] = nc.declare_dram_parameter("wemb1", [128, D], bf, isOutput=False)
    P["wemb2"] = nc.declare_dram_parameter("wemb2", [DIN - 128, D], bf, isOutput=False)
    P["b_emb"] = nc.declare_dram_parameter("b_emb", [D, 1], f32, isOutput=False)
    P["W_gcn"] = nc.declare_dram_parameter("W_gcn", [L, D, D], bf, isOutput=False)
    P["gamma_t"] = nc.declare_dram_parameter("gamma_t", [D, L], f32, isOutput=False)
    P["beta_t"] = nc.declare_dram_parameter("beta_t", [D, L], f32, isOutput=False)
    P["W_r1"] = nc.declare_dram_parameter("W_r1", [D, D // 2], f32, isOutput=False)
    P["b_r1"] = nc.declare_dram_parameter("b_r1", [D // 2, 1], f32, isOutput=False)
    P["W_r2"] = nc.declare_dram_parameter("W_r2", [D // 2, D // 4], f32, isOutput=False)
    P["b_r2"] = nc.declare_dram_parameter("b_r2", [D // 4, 1], f32, isOutput=False)
    P["W_r3"] = nc.declare_dram_parameter("W_r3", [D // 4, NC], f32, isOutput=False)
    P["b_r3"] = nc.declare_dram_parameter("b_r3", [NC, 1], f32, isOutput=False)
    P["ident"] = nc.declare_dram_parameter("ident", [128, 128], f32, isOutput=False)
    P["identb"] = nc.declare_dram_parameter("identb", [128, 128], bf, isOutput=False)
    P["inv_cnt"] = nc.declare_dram_parameter("inv_cnt", [G, 1], f32, isOutput=False)
    out_p = nc.declare_dram_parameter("out", [NC, G], f32, isOutput=True)

    rg = [list(range(C))]

    with tile.TileContext(nc) as tc:
        with (
            tc.tile_pool(name="const", bufs=1) as cst,
            tc.tile_pool(name="hbuf", bufs=1) as hbuf,
            tc.tile_pool(name="gd", bufs=6) as gd,
            tc.tile_pool(name="sp", bufs=6) as sp,
            tc.tile_pool(name="work", bufs=3) as wk,
            tc.tile_pool(name="xst", bufs=1) as xst,
            tc.tile_pool(name="pag", bufs=4, space="PSUM") as pag,
            tc.tile_pool(name="pmm", bufs=2, space="PSUM") as pmm,
            tc.tile_pool(name="pmo", bufs=1, space="PSUM") as pmo,
            tc.tile_pool(name="dram", bufs=1, space="DRAM") as drp,
        ):
            def load_const(name, shape, dt=f32):
                t = cst.tile(shape, dt, tag=f"c_{name}")
                nc.sync.dma_start(out=t[:], in_=P[name][:])
                return t

            idx_sb = load_const("idx", [128, IDXCOLS], i16)
            sd_sb = load_const("sd", [128, NT * 128], bf)
            pm_sb = load_const("Pm", [128, NT * G], bf)
            dinv_sb = load_const("dinv_p", [128, NT])
            wemb1 = load_const("wemb1", [128, D], bf)
            wemb2 = load_const("wemb2", [DIN - 128, D], bf)
            bemb_sb = load_const("b_emb", [D, 1])
            gamma_sb = load_const("gamma_t", [D, L])
            beta_sb = load_const("beta_t", [D, L])
            wr1_sb = load_const("W_r1", [D, D // 2])
            br1_sb = load_const("b_r1", [D // 2, 1])
            wr2_sb = load_const("W_r2", [D // 2, D // 4])
            br2_sb = load_const("b_r2", [D // 4, 1])
            wr3_sb = load_const("W_r3", [D // 4, NC])
            br3_sb = load_const("b_r3", [NC, 1])
            ident_sb = load_const("ident", [128, 128])
            identb_sb = load_const("identb", [128, 128], bf)
            invc_sb = load_const("inv_cnt", [G, 1])
            wgcn_sb = cst.tile([128, L * D], bf)
            for l in range(L):
                nc.sync.dma_start(out=wgcn_sb[:, l * D:(l + 1) * D],
                                  in_=P["W_gcn"][l])

            hA = hbuf.tile([128, NT * 128], bf)
            hB = hbuf.tile([128, NT * 128], bf)
            hagg = hbuf.tile([128, NT * 128], f32)
            m_sb = hbuf.tile([128, NT * 128], bf)
            sums = hbuf.tile([128, NT], f32)
            sumsq = hbuf.tile([128, NT], f32)

            m_bounceA = drp.tile([HLOC, D], bf)
            m_bounceB = drp.tile([NL - HLOC, D], bf)
            m_fullA = [drp.tile([HA, D], bf, name=f"m_fullA{l}")
                       for l in range(L)]
            m_fullB = [drp.tile([HB, D], bf, name=f"m_fullB{l}")
                       for l in range(L)]
            stat_in = drp.tile([128, 2], f32)
            stat_out = [drp.tile([128, 2], f32, name=f"stat_out{l}")
                        for l in range(L)]
            pool_in = drp.tile([G, D], f32)
            pool_out = drp.tile([G, D], f32)

            # ---- embedding: h0_T = W_emb.T @ x_T + b_emb (x resident) ----
            x1_sb = xst.tile([128, NL], bf, tag="x1")
            nc.sync.dma_start(out=x1_sb[:], in_=P["x1_t"][:])
            x2_sb = xst.tile([DIN - 128, NL], bf, tag="x2")
            nc.sync.dma_start(out=x2_sb[:], in_=P["x2_t"][:])
            hbufs = [hA, hB]
            qn = [0]

            def m_phase_tile(h_src, l, t):
                """m = dinv * (h @ W_l) for node tile t: PSUM -> bf16 m_sb ->
                bounce-half DMA; triggers the half AllGathers at t=TA-1/NT-1."""
                cw = NT_W[t]
                W_l = wgcn_sb[:, l * D:(l + 1) * D]
                pm = pmm.tile([128, D], f32, tag="pm", name="pm")
                nc.tensor.matmul(out=pm[:cw, :],
                                 lhsT=h_src[:, t * 128:t * 128 + cw],
                                 rhs=W_l, start=True, stop=True)
                nc.scalar.activation(out=m_sb[:cw, t * D:(t + 1) * D],
                                     in_=pm[:cw, :], func=Act.Identity,
                                     bias=0.0, scale=dinv_sb[:cw, t:t + 1])
                if t < TA:
                    nc.sync.dma_start(
                        out=m_bounceA[t * 128:t * 128 + cw, :],
                        in_=m_sb[:cw, t * D:(t + 1) * D])
                else:
                    r0 = (t - TA) * 128
                    nc.sync.dma_start(
                        out=m_bounceB[r0:r0 + cw, :],
                        in_=m_sb[:cw, t * D:(t + 1) * D])
                if t == TA - 1:
                    nc.gpsimd.collective_compute(
                        "AllGather", Alu.bypass, replica_groups=rg,
                        ins=[m_bounceA.opt()], outs=[m_fullA[l].opt()])
                if t == NT - 1:
                    nc.gpsimd.collective_compute(
                        "AllGather", Alu.bypass, replica_groups=rg,
                        ins=[m_bounceB.opt()], outs=[m_fullB[l].opt()])

            # embedding + layer-0 m fused per tile
            for t in range(NT):
                c0 = t * 128
                cw = NT_W[t]
                pe = pmm.tile([128, 128], f32, tag="pm", name="pe")
                nc.tensor.matmul(out=pe[:, :cw], lhsT=wemb1[:],
                                 rhs=x1_sb[:, c0:c0 + cw], start=True, stop=False)
                nc.tensor.matmul(out=pe[:, :cw], lhsT=wemb2[:],
                                 rhs=x2_sb[:, c0:c0 + cw], start=False, stop=True)
                nc.scalar.activation(out=hA[:, c0:c0 + cw], in_=pe[:, :cw],
                                     func=Act.Identity, bias=bemb_sb[:, 0:1],
                                     scale=1.0)
                m_phase_tile(hA, 0, t)

            ppool = pmo.tile([G, D], f32, tag="ppool")

            # ---- GCN layers ----
            for l in range(L):
                h_in = hbufs[l % 2]
                h_out = hbufs[(l + 1) % 2]

                # issue all gathers + S streams (pipelined via pool bufs)
                gts = {}
                sgs = {}
                for ci, (h, t0, j, off) in enumerate(calls):
                    gt = gd.tile([128, MAXJ, D], bf, tag="gt")
                    tab = m_fullA[l] if h == 0 else m_fullB[l]
                    nc.gpsimd.dma_gather(
                        gt[:, :j, :], tab[:], idx_sb[:, off:off + 8 * j],
                        128 * j, 128 * j, D, queue_num=qn[0] % 4)
                    qn[0] += 1
                    sg = sp.tile([128, MAXJ * D], bf, tag="sg")
                    nc.sync.dma_start(out=sg[:, :j * D],
                                      in_=P["S"][:, t0 * D:(t0 + j) * D])
                    for jj in range(j):
                        gts[t0 + jj] = (gt, jj)
                        sgs[t0 + jj] = (sg, jj)

                # aggregate per window: seed opens PSUM, edge tiles accumulate
                for ws in groups:
                    for w in ws:
                        cw = NT_W[w]
                        tiles_w = (list(range(tile_base[(w, 0)],
                                              tile_base[(w, 0)] + int(T[w, 0])))
                                   + list(range(tile_base[(w, 1)],
                                                tile_base[(w, 1)] + int(T[w, 1]))))
                        pw = pag.tile([128, 128], f32, tag="pw")
                        nc.tensor.matmul(
                            out=pw[:], lhsT=m_sb[:cw, w * D:(w + 1) * D],
                            rhs=sd_sb[:cw, w * 128:(w + 1) * 128],
                            start=True, stop=(len(tiles_w) == 0))
                        for i, ti in enumerate(tiles_w):
                            gt, jj = gts[ti]
                            sg, js = sgs[ti]
                            nc.tensor.matmul(
                                out=pw[:], lhsT=gt[:, jj, :],
                                rhs=sg[:, js * D:(js + 1) * D],
                                start=False, stop=(i == len(tiles_w) - 1))
                        w0 = w * 128
                        nc.vector.tensor_copy(out=hagg[:, w0:w0 + cw],
                                              in_=pw[:, :cw])
                        nc.vector.reduce_sum(out=sums[:, w:w + 1],
                                             in_=hagg[:, w0:w0 + cw], axis=Axis.X)
                        sq = wk.tile([128, 128], f32, tag="sq")
                        nc.scalar.square(out=sq[:, :cw], in_=hagg[:, w0:w0 + cw])
                        nc.vector.reduce_sum(out=sumsq[:, w:w + 1],
                                             in_=sq[:, :cw], axis=Axis.X)

                # global BN stats (b_gcn cancels inside BN and is dropped)
                stat_sb = wk.tile([128, 2], f32, tag="stat")
                nc.vector.reduce_sum(out=stat_sb[:, 0:1], in_=sums[:], axis=Axis.X)
                nc.vector.reduce_sum(out=stat_sb[:, 1:2], in_=sumsq[:], axis=Axis.X)
                nc.sync.dma_start(out=stat_in[:], in_=stat_sb[:])
                nc.gpsimd.collective_compute(
                    "AllReduce", Alu.add, replica_groups=rg,
                    ins=[stat_in.opt()], outs=[stat_out[l].opt()])
                stat_g = wk.tile([128, 2], f32, tag="statg")
                nc.sync.dma_start(out=stat_g[:], in_=stat_out[l][:])

                mu = wk.tile([128, 1], f32, tag="mu")
                nc.vector.tensor_scalar(out=mu[:], in0=stat_g[:, 0:1],
                                        scalar1=1.0 / N, scalar2=None, op0=Alu.mult)
                ex2 = wk.tile([128, 1], f32, tag="ex2")
                nc.vector.tensor_scalar(out=ex2[:], in0=stat_g[:, 1:2],
                                        scalar1=1.0 / N, scalar2=None, op0=Alu.mult)
                musq = wk.tile([128, 1], f32, tag="musq")
                nc.vector.tensor_tensor(out=musq[:], in0=mu[:], in1=mu[:], op=Alu.mult)
                var = wk.tile([128, 1], f32, tag="var")
                nc.vector.tensor_tensor(out=var[:], in0=ex2[:], in1=musq[:],
                                        op=Alu.subtract)
                var2 = wk.tile([128, 1], f32, tag="var2")
                nc.vector.tensor_scalar(out=var2[:], in0=var[:], scalar1=float(EPS),
                                        scalar2=None, op0=Alu.add)
                stdv = wk.tile([128, 1], f32, tag="stdv")
                nc.scalar.activation(out=stdv[:], in_=var2[:], func=Act.Sqrt)
                rinv = wk.tile([128, 1], f32, tag="rinv")
                nc.vector.reciprocal(out=rinv[:], in_=stdv[:])
                a_t = wk.tile([128, 1], f32, tag="a_t")
                nc.vector.tensor_tensor(out=a_t[:], in0=gamma_sb[:, l:l + 1],
                                        in1=rinv[:], op=Alu.mult)
                t1 = wk.tile([128, 1], f32, tag="t1")
                nc.vector.tensor_tensor(out=t1[:], in0=mu[:], in1=a_t[:], op=Alu.mult)
                b2 = wk.tile([128, 1], f32, tag="b2")
                nc.vector.tensor_tensor(out=b2[:], in0=beta_sb[:, l:l + 1],
                                        in1=t1[:], op=Alu.subtract)

                # fused BN apply + residual + next-layer m (or final pool)
                for t in range(NT):
                    cw = NT_W[t]
                    w0 = t * 128
                    rl = wk.tile([128, 128], bf, tag="rl")
                    nc.scalar.activation(out=rl[:, :cw], in_=hagg[:, w0:w0 + cw],
                                         func=Act.Relu, bias=b2[:, 0:1],
                                         scale=a_t[:, 0:1])
                    nc.vector.tensor_add(out=h_out[:, w0:w0 + cw],
                                         in0=rl[:, :cw], in1=h_in[:, w0:w0 + cw])
                    if l < L - 1:
                        m_phase_tile(h_out, l + 1, t)
                    else:
                        pt = pmm.tile([128, 128], f32, tag="pm", name="pt")
                        nc.tensor.matmul(out=pt[:cw, :],
                                         lhsT=h_out[:, w0:w0 + cw],
                                         rhs=identb_sb[:], start=True, stop=True)
                        hr = wk.tile([128, 128], bf, tag="hr")
                        nc.scalar.activation(out=hr[:cw, :], in_=pt[:cw, :],
                                             func=Act.Identity, bias=0.0, scale=1.0)
                        nc.tensor.matmul(out=ppool[:],
                                         lhsT=pm_sb[:cw, t * G:(t + 1) * G],
                                         rhs=hr[:cw, :],
                                         start=(t == 0), stop=(t == NT - 1))

            # ---- MLP readout ----
            pool_sb = wk.tile([G, D], f32, tag="pool_sb")
            nc.vector.tensor_copy(out=pool_sb[:], in_=ppool[:])
            nc.sync.dma_start(out=pool_in[:], in_=pool_sb[:])
            nc.gpsimd.collective_compute(
                "AllReduce", Alu.add, replica_groups=rg,
                ins=[pool_in.opt()], outs=[pool_out.opt()])
            pg = wk.tile([G, D], f32, tag="pg")
            nc.sync.dma_start(out=pg[:], in_=pool_out[:])
            hg = wk.tile([G, D], f32, tag="hg")
            nc.vector.tensor_scalar(out=hg[:], in0=pg[:], scalar1=invc_sb[:, 0:1],
                                    scalar2=None, op0=Alu.mult)
            pt2 = pmo.tile([128, G], f32, tag="pmlp")
            nc.tensor.matmul(out=pt2[:], lhsT=hg[:], rhs=ident_sb[:G, :G],
                             start=True, stop=True)
            hgT = wk.tile([128, G], f32, tag="hgT")
            nc.vector.tensor_copy(out=hgT[:], in_=pt2[:])
            pz1 = pmo.tile([D // 2, G], f32, tag="pmlp")
            nc.tensor.matmul(out=pz1[:], lhsT=wr1_sb[:], rhs=hgT[:],
                             start=True, stop=True)
            z1 = wk.tile([D // 2, G], f32, tag="z1")
            nc.scalar.activation(out=z1[:], in_=pz1[:], func=Act.Relu,
                                 bias=br1_sb[:, 0:1], scale=1.0)
            pz2 = pmo.tile([D // 4, G], f32, tag="pmlp")
            nc.tensor.matmul(out=pz2[:], lhsT=wr2_sb[:], rhs=z1[:],
                             start=True, stop=True)
            z2 = wk.tile([D // 4, G], f32, tag="z2")
            nc.scalar.activation(out=z2[:], in_=pz2[:], func=Act.Relu,
                                 bias=br2_sb[:, 0:1], scale=1.0)
            pz3 = pmo.tile([NC, G], f32, tag="pmlp")
            nc.tensor.matmul(out=pz3[:], lhsT=wr3_sb[:], rhs=z2[:],
                             start=True, stop=True)
            z3 = wk.tile([NC, G], f32, tag="z3")
            nc.scalar.activation(out=z3[:], in_=pz3[:], func=Act.Identity,
                                 bias=br3_sb[:, 0:1], scale=1.0)
            nc.sync.dma_start(out=out_p[:], in_=z3[:])

    nc.compile()
    return nc


_CACHE = {}


def kernel(x, edge_index, batch, W_emb, b_emb, W_gcn, b_gcn,
           bn_gamma, bn_beta, W_r1, b_r1, W_r2, b_r2, W_r3, b_r3):
    import os
    from concourse.bass_utils import run_bass_kernel_spmd
    trace = bool(os.environ.get("BASS_KERNEL_TRACE"))

    x = np.asarray(x, dtype=np.float32)
    meta, per_core, inv_cnt, bf16 = _preprocess(x, edge_index, batch)

    key = (meta["TILES"], meta["IDXCOLS"], tuple(map(tuple, meta["T"])))
    if key not in _CACHE:
        _CACHE[key] = _build(meta)
    nc = _CACHE[key]

    W_emb = np.asarray(W_emb, np.float32)
    shared = dict(
        wemb1=np.ascontiguousarray(W_emb[:128]).astype(bf16),
        wemb2=np.ascontiguousarray(W_emb[128:]).astype(bf16),
        b_emb=np.asarray(b_emb, np.float32).reshape(D, 1),
        W_gcn=np.asarray(W_gcn, np.float32).astype(bf16),
        gamma_t=np.ascontiguousarray(np.asarray(bn_gamma, np.float32).T),
        beta_t=np.ascontiguousarray(np.asarray(bn_beta, np.float32).T),
        W_r1=np.asarray(W_r1, np.float32),
        b_r1=np.asarray(b_r1, np.float32).reshape(D // 2, 1),
        W_r2=np.asarray(W_r2, np.float32),
        b_r2=np.asarray(b_r2, np.float32).reshape(D // 4, 1),
        W_r3=np.asarray(W_r3, np.float32),
        b_r3=np.asarray(b_r3, np.float32).reshape(NC, 1),
        ident=np.eye(128, dtype=np.float32),
        identb=np.eye(128, dtype=np.float32).astype(bf16),
        inv_cnt=inv_cnt,
    )
    in_maps = []
    for c in range(C):
        m = dict(shared)
        m.update(per_core[c])
        in_maps.append(m)

    res = run_bass_kernel_spmd(nc, in_maps, core_ids=list(range(C)), trace=trace)
    if trace and res.exec_time_ns is not None:
        print(f"HW exec time: {res.exec_time_ns} ns")
    out = res.results[0]["out"]  # [NC, G]
    return np.ascontiguousarray(out.T.astype(np.float32))  # [G, NC]
